# revision 41
# baseline (speedup 1.0000x reference)
"""Multi-head causal attention (B=2, S=2048, D=1024, H=16, hd=64) on 8 trn2 cores.

Sharding: core c handles batch b = c//4 and head-group g = c%4 (heads 4g..4g+4,
d-slice 256g..256g+256 of the QKV projections / Wo rows).  Each core computes a
partial out-projection [2048, 1024] in bf16; the host sums the 4 head-group
partials per batch in f32 and adds the bias.

Per-core kernel (all matmuls bf16, accumulate f32 in PSUM):
  qT/kT = (x @ Wq/k)^T computed directly as [256, 2048] via lhsT=W chunks.
  v     = x @ Wv in natural [seq, head, 66] layout (cols 64/65 = 1.0 so the
          attention rowsum falls out of the ctx matmul as row 64).
  S^T   = k_h @ q_h^T  [kpos, qpos] tiles; exp via ACT (scale=1/8) PSUM->SBUF;
          causal = skip invalid column blocks + triangular bf16 mask on
          diagonal blocks.
  ctx~T = v'_h^T @ expS^T accumulated over kpos blocks -> [66, 512] PSUM
          (row 64 = softmax denominator).
  out  += (ctx~T / rowsum)^T @ Wo rows (normalization: approx reciprocal +
          SBUF->SBUF partition-broadcast DMA + DVE multiply, with the
          multiplies emitted at the start of the following group so the DMA
          latency never blocks the DVE queue).

Scheduling: host prepacks x into contiguous [j][k][128, 512] blocks so each of
the 4 column-group DMAs is one big issue landing j=0 first; weights are one
issue each.  Emission is software-pipelined: attention groups run in ascending
size right behind their projections, with projection / out-projection matmuls
pulled from a fill queue between each score->exp->ctx step so the PE never
waits on the ACT exp.  PSUM->SBUF copies are spread over DVE and GpSimd; ACT
does only exp.
"""

import sys

import numpy as np

for _p in ("/opt/trn_rl_repo",):
    if _p not in sys.path:
        sys.path.insert(0, _p)

import ml_dtypes

import concourse.bass as bass
import concourse.mybir as mybir
import concourse.tile as tile
from concourse import bacc
from concourse.bass_utils import run_bass_kernel_spmd
from concourse.masks import make_upper_triangular

BF16 = mybir.dt.bfloat16
F32 = mybir.dt.float32

import os

NRM_PE = os.environ.get("K_NRM_PE", "1") == "1"      # PE-broadcast normalize
# NOTE: K-split ctx pairs hang real hardware (a PSUM accumulation group may
# not span two PE tile positions), though CoreSim/TimelineSim accept them.
CTX_SPLIT = os.environ.get("K_CTX_SPLIT", "0") == "1"  # K-split paired ctx
W_ACT_RING = os.environ.get("K_W_ACT", "1") == "1"   # weights on ACT hwdge ring
SP_TAIL = os.environ.get("K_SP_TAIL", "1") == "1"    # tail outproj via spool

B, S, D, H, HD = 2, 2048, 1024, 16, 64
NCORES = 8
HPC = 4          # heads per core
DHC = HPC * HD   # 256: d-slice per core
P = 128
SB = S // P      # 16 seq blocks
KC = D // P      # 8 contraction chunks for projections
QG = 512         # q column group width
NQG = S // QG    # 4
VW = HD + 2      # 66: v cols per head (64 data + 2 ones; even M for PE)


def _build_body(ctx, tc, io):
    nc = tc.nc
    xb, wq, wk, wv, wo, out = (
        io["xb"], io["wq"], io["wk"], io["wv"], io["wo"], io["out"],
    )

    consts = ctx.enter_context(tc.tile_pool(name="consts", bufs=1))
    persist = ctx.enter_context(tc.tile_pool(name="persist", bufs=1))
    spool = ctx.enter_context(tc.tile_pool(name="spsum", bufs=2, space="PSUM"))
    cxpool = ctx.enter_context(tc.tile_pool(name="cxpsum", bufs=2, space="PSUM"))
    pjpool = ctx.enter_context(tc.tile_pool(name="pjpsum", bufs=2, space="PSUM"))
    espool = ctx.enter_context(tc.tile_pool(name="es", bufs=6))
    nrmpool = ctx.enter_context(tc.tile_pool(name="nrm", bufs=4))
    outpool = ctx.enter_context(tc.tile_pool(name="outsb", bufs=3))

    # triangular keep-mask for diagonal blocks: tri[i, j] = 1.0 iff j >= i
    tri = consts.tile([P, P], BF16, tag="tri", name="tri")
    make_upper_triangular(nc, tri[:], val=1.0, diag=True)

    # [1, 64] ones: lhsT of the rowsum partition-broadcast matmul
    ones_l = consts.tile([1, HD], BF16, tag="ones", name="ones")
    nc.vector.memset(ones_l[:], 1.0)

    if not NRM_PE:
        drampool = ctx.enter_context(tc.tile_pool(name="dram", bufs=1,
                                                  space="DRAM"))
        rsc = drampool.tile([16, QG], F32, tag="rsc", name="rsc")

    # preload the ACT exp table so the first real exp doesn't pay the load
    dumm = consts.tile([1, 2], F32, tag="dumm", name="dumm")
    nc.vector.memset(dumm[:], 0.0)
    nc.scalar.activation(dumm[:], dumm[:],
                         mybir.ActivationFunctionType.Exp, scale=0.125)

    # ---- persistent SBUF tensors ----
    # x column-groups: xt[j] holds all 8 k-chunks of q-columns j*512..+512
    xt = [persist.tile([P, KC, QG], BF16, tag=f"xt{j}", name=f"xt{j}")
          for j in range(NQG)]
    wq_sb = persist.tile([P, KC, DHC], BF16, tag="wq", name="wq")
    wk_sb = persist.tile([P, KC, DHC], BF16, tag="wk", name="wk")
    wv_sb = persist.tile([P, KC, DHC], BF16, tag="wv", name="wv")
    wo_sb = persist.tile([P, 2, D], BF16, tag="wo", name="wo")
    v_sb = [persist.tile([P, HPC, VW], BF16, tag=f"v{s}", name=f"v{s}")
            for s in range(SB)]
    qt = [persist.tile([P, S], BF16, tag=f"qt{i}", name=f"qt{i}") for i in range(2)]
    kt = [persist.tile([P, S], BF16, tag=f"kt{i}", name=f"kt{i}") for i in range(2)]
    ctxT = [persist.tile([P, S], BF16, tag=f"ctxT{i}", name=f"ctxT{i}")
            for i in range(2)]

    # v ones columns are constant: set once up front (cols 64,65 both 1.0;
    # col 65 is never read, it only keeps M even for the PE)
    for s in range(SB):
        nc.vector.memset(v_sb[s][:, :, HD:VW], 1.0)

    # ---- input DMAs: few big issues, first-needed data first ----
    def _w_src(dram, kc, dhc):
        ap = dram[:, :]
        return bass.AP(ap.tensor, ap.offset, [[dhc, P], [P * dhc, kc], [1, dhc]])

    def _x_src(j):
        ap = xb[:, :]
        return bass.AP(ap.tensor, ap.offset + j * (KC * P * QG),
                       [[QG, P], [P * QG, KC], [1, QG]])

    # weights on the Activation hwdge ring, x on the SP ring: the two issue
    # streams overlap so first-needed data lands sooner.  Only the first-needed
    # transfers are issued up front; the rest are issued later in the stream so
    # they don't steal DMA bandwidth from the critical first tiles.
    weng = nc.scalar if W_ACT_RING else nc.sync
    weng.dma_start(out=wv_sb[:], in_=_w_src(wv, KC, DHC))
    nc.sync.dma_start(out=xt[0][:], in_=_x_src(0))
    weng.dma_start(out=wq_sb[:], in_=_w_src(wq, KC, DHC))
    weng.dma_start(out=wk_sb[:], in_=_w_src(wk, KC, DHC))

    def emit_late_dmas():
        # issued on the ACT ring AFTER the first v-proj copy is queued there:
        # the copy's data dependency stalls the ACT queue until the first
        # matmuls run, so these transfers can't steal DMA bandwidth from the
        # critical wv/wq/wk/xt0 tiles above
        nc.scalar.dma_start(out=xt[1][:], in_=_x_src(1))
        nc.scalar.dma_start(out=xt[2][:], in_=_x_src(2))
        nc.scalar.dma_start(out=xt[3][:], in_=_x_src(3))
        nc.scalar.dma_start(out=wo_sb[:], in_=_w_src(wo, 2, D))

    # ---- fill-unit generators (each yield = ~1-2 PE matmuls emitted) ----
    def gen_v(sv):
        # seq blocks s=2sv, 2sv+1 -> v natural layout
        for par in range(2):
            s = 2 * sv + par
            j, sj = divmod(s, 4)
            ps = pjpool.tile([P, QG], F32, tag="pj", name="pj")
            for k in range(KC):
                nc.tensor.matmul(
                    ps[:, 0:DHC],
                    lhsT=xt[j][:, k, sj * P:(sj + 1) * P],
                    rhs=wv_sb[:, k, :],
                    start=(k == 0),
                    stop=(k == KC - 1),
                )
                if k % 2 == 1:
                    yield
            src_ap = ps[:, 0:DHC].rearrange("p (h d) -> p h d", h=HPC)
            if par == 0:
                nc.scalar.copy(v_sb[s][:, :, 0:HD], src_ap)
            else:
                nc.vector.tensor_copy(v_sb[s][:, :, 0:HD], src_ap)
            yield

    def gen_qk(pair, j):
        # q and k projections for d-chunk `pair`, q column group j
        for w_sb, dst, use_act in ((wq_sb, qt, False), (wk_sb, kt, True)):
            ps = pjpool.tile([P, QG], F32, tag="pj", name="pj")
            for k in range(KC):
                nc.tensor.matmul(
                    ps[:],
                    lhsT=w_sb[:, k, pair * P:(pair + 1) * P],
                    rhs=xt[j][:, k, :],
                    start=(k == 0),
                    stop=(k == KC - 1),
                )
                if k % 2 == 1:
                    yield
            if use_act:
                nc.scalar.copy(dst[pair][:, j * QG:(j + 1) * QG], ps[:])
            else:
                nc.vector.tensor_copy(dst[pair][:, j * QG:(j + 1) * QG], ps[:])
            yield

    def gen_outproj(m, use_sp=False):
        ot = outpool.tile([P, D], BF16, tag="ot", name="ot")
        if use_sp:
            # tail variant: attention is done, so use the (free) score pool —
            # its 2 banks hold both 512-col halves, and bufs=2 pipelines
            # consecutive m's
            ps = spool.tile([P, 2, QG], F32, tag="sp", name="sp")
            for kc in range(2):
                mb = ctxT[kc][:, m * P:(m + 1) * P]
                st, sp_ = (kc == 0), (kc == 1)
                nc.tensor.matmul(ps[:, 0, :], lhsT=mb,
                                 rhs=wo_sb[:, kc, 0:QG], start=st, stop=sp_)
                nc.tensor.matmul(ps[:, 1, :], lhsT=mb,
                                 rhs=wo_sb[:, kc, QG:D], start=st, stop=sp_)
                yield
            nc.scalar.copy(ot[:, 0:QG], ps[:, 0, :])
            nc.vector.tensor_copy(ot[:, QG:D], ps[:, 1, :])
            yield
        else:
            for n2 in range(2):
                ps = pjpool.tile([P, QG], F32, tag="pj", name="pj")
                for kc in range(2):
                    nc.tensor.matmul(
                        ps[:],
                        lhsT=ctxT[kc][:, m * P:(m + 1) * P],
                        rhs=wo_sb[:, kc, n2 * QG:(n2 + 1) * QG],
                        start=(kc == 0),
                        stop=(kc == 1),
                    )
                yield
                nc.vector.tensor_copy(ot[:, n2 * QG:(n2 + 1) * QG], ps[:])
                yield
        nc.sync.dma_start(out=out[m * P:(m + 1) * P, :], in_=ot[:])
        yield

    # ---- fill queue machinery (purely an emission-order device) ----
    queue = []           # list of (name, generator)
    done = set()

    def push(name, gen):
        queue.append((name, gen))

    def pull(n):
        emitted = 0
        while emitted < n and queue:
            name, gen = queue[0]
            try:
                next(gen)
                emitted += 1
            except StopIteration:
                done.add(name)
                queue.pop(0)

    def drain_until(name):
        # emit exactly up to the end of generator `name` (not the whole queue)
        while name not in done:
            if not queue:
                raise RuntimeError(f"drain_until({name}): not queued")
            pull(1)

    def finish_head():
        # run the head generator to completion so no fill generator holds an
        # in-flight pjpool tile (bufs=1) when we emit pj work directly —
        # interleaving would deadlock the PE queue on the shared bank
        if queue:
            name = queue[0][0]
            while queue and queue[0][0] == name:
                pull(1)

    # deferred softmax-normalize multiplies: (pair, g, cxs, rb) from the
    # previous attention group, emitted shortly into the next one (gives the
    # broadcast DMA time to land without blocking the DVE queue); finished
    # out-projection inputs are queued as fill right after
    pending_nrm = [None]

    def emit_pending_nrm():
        if pending_nrm[0] is None:
            return
        pair, g, cxs, rsb = pending_nrm[0]
        pending_nrm[0] = None
        rbr = nrmpool.tile([HD, 2, QG], F32, tag="rbr", name="rbr")
        if NRM_PE:
            finish_head()
            for hh in range(2):
                # broadcast the bf16 rowsum over 64 partitions via a K=1
                # matmul, then reciprocal straight off PSUM and normalize
                bc = pjpool.tile([P, QG], F32, tag="pj", name="pj")
                nc.tensor.matmul(bc[0:HD, :], lhsT=ones_l[:], rhs=rsb[hh][:],
                                 start=True, stop=True)
                nc.vector.reciprocal_approx_fast(rbr[:, hh, :], bc[0:HD, :])
                nc.vector.tensor_mul(
                    ctxT[pair][hh * HD:(hh + 1) * HD, g * QG:(g + 1) * QG],
                    cxs[hh][0:HD, :],
                    rbr[:, hh, :],
                )
        else:
            rb = rsb  # [HD, 2, QG] broadcast rowsums read back from DRAM
            nc.vector.reciprocal_approx_fast(rbr[:], rb[:])
            for hh in range(2):
                nc.vector.tensor_mul(
                    ctxT[pair][hh * HD:(hh + 1) * HD, g * QG:(g + 1) * QG],
                    cxs[hh][0:HD, :],
                    rbr[:, hh, :],
                )
        if pair == 1:
            for m in range(4 * g, 4 * g + 4):
                push(f"op{m}", gen_outproj(m, use_sp=(g == 3 and SP_TAIL)))

    # ---- attention group ----
    def emit_attention_group(pair, g, npull):
        cxs = [cxpool.tile([VW, QG], F32, tag="cx", name="cx") for _ in range(2)]
        nkb = 4 * g + 4
        sp_tiles = {}

        def emit_score(kb):
            c0 = P * (kb - 4 * g) if kb >= 4 * g else 0
            sp_t = spool.tile([P, 2, QG], F32, tag="sp", name="sp")
            for hh in range(2):
                nc.tensor.matmul(
                    sp_t[:, hh, c0:QG],
                    lhsT=kt[pair][hh * HD:(hh + 1) * HD, kb * P:(kb + 1) * P],
                    rhs=qt[pair][hh * HD:(hh + 1) * HD, g * QG + c0:(g + 1) * QG],
                    start=True,
                    stop=True,
                )
            sp_tiles[kb] = (sp_t, c0)

        emit_score(0)
        for kb in range(nkb):
            sp_t, c0 = sp_tiles.pop(kb)
            es_t = espool.tile([P, 2, QG], BF16, tag="es", name="es")
            nc.scalar.activation(
                es_t[:, :, c0:QG], sp_t[:, :, c0:QG],
                mybir.ActivationFunctionType.Exp, scale=0.125,
            )
            if kb >= 4 * g:
                dst = es_t[:, :, c0:c0 + P]
                t_ap = tri[:]
                tri_b = bass.AP(t_ap.tensor, t_ap.offset,
                                [t_ap.ap[0], [0, 2], t_ap.ap[1]])
                nc.vector.tensor_mul(dst, dst, tri_b)
            if kb == 0:
                emit_pending_nrm()
            if kb + 1 < nkb:
                emit_score(kb + 1)
            pull(npull)
            st = (kb == 0)
            sp_ = (kb == nkb - 1)
            h0, h1 = 2 * pair, 2 * pair + 1
            if CTX_SPLIT:
                # K split in 64-row halves so the two heads run concurrently
                # on the two PE row-quadrants (different PSUM banks)
                nc.tensor.matmul(cxs[0][:, c0:QG], lhsT=v_sb[kb][0:HD, h0, :],
                                 rhs=es_t[0:HD, 0, c0:QG], start=st, stop=False)
                nc.tensor.matmul(cxs[1][:, c0:QG], lhsT=v_sb[kb][HD:P, h1, :],
                                 rhs=es_t[HD:P, 1, c0:QG], start=st, stop=False)
                nc.tensor.matmul(cxs[0][:, c0:QG], lhsT=v_sb[kb][HD:P, h0, :],
                                 rhs=es_t[HD:P, 0, c0:QG], start=False, stop=sp_)
                nc.tensor.matmul(cxs[1][:, c0:QG], lhsT=v_sb[kb][0:HD, h1, :],
                                 rhs=es_t[0:HD, 1, c0:QG], start=False, stop=sp_)
            else:
                for hh in range(2):
                    h = 2 * pair + hh
                    nc.tensor.matmul(
                        cxs[hh][:, c0:QG],
                        lhsT=v_sb[kb][:, h, :],
                        rhs=es_t[:, hh, c0:QG],
                        start=st,
                        stop=sp_,
                    )
        # group end: stage the rowsum rows; broadcast + reciprocal +
        # normalize deferred to the next group
        if NRM_PE:
            rsb = [nrmpool.tile([1, QG], BF16, tag=f"rs{hh}", name=f"rs{hh}")
                   for hh in range(2)]
            for hh in range(2):
                nc.vector.tensor_copy(rsb[hh][:], cxs[hh][HD:HD + 1, :])
            pending_nrm[0] = (pair, g, cxs, rsb)
        else:
            slot = (pair * NQG + g) * 2
            for hh in range(2):
                rs = nrmpool.tile([1, QG], F32, tag=f"rs{hh}", name=f"rs{hh}")
                nc.vector.tensor_copy(rs[:], cxs[hh][HD:HD + 1, :])
                nc.sync.dma_start(out=rsc[slot + hh:slot + hh + 1, :],
                                  in_=rs[:])
            rb = nrmpool.tile([HD, 2, QG], F32, tag="rb", name="rb")
            sl = rsc[slot:slot + 2, :]
            src = bass.AP(sl.tensor, sl.offset, [[0, HD], [QG, 2], [1, QG]])
            nc.sync.dma_start(out=rb[:], in_=src)
            pending_nrm[0] = (pair, g, cxs, rb)

    # ---- emission schedule ----
    # v projections for the first column group, then qk(0,0), then attention
    # groups in ascending size with everything else pulled in as filler.
    for _ in gen_v(0):
        pass
    emit_late_dmas()
    for _ in gen_v(1):
        pass
    for _ in gen_qk(0, 0):
        pass
    for j in range(1, NQG):
        push(f"v{2 * j}", gen_v(2 * j))
        push(f"v{2 * j + 1}", gen_v(2 * j + 1))
        push(f"qk0{j}", gen_qk(0, j))
    for j in range(NQG):
        push(f"qk1{j}", gen_qk(1, j))

    for g in range(NQG):
        if g > 0:
            drain_until(f"qk0{g}")
        emit_attention_group(0, g, npull=3)
    for g in range(NQG):
        drain_until(f"qk1{g}")
        emit_attention_group(1, g, npull=4)
    emit_pending_nrm()
    pull(1 << 30)


def build_nc():
    from contextlib import ExitStack

    nc = bacc.Bacc()
    io = {
        "xb": nc.dram_tensor("xb", [NQG, KC, P, QG], BF16,
                             kind="ExternalInput").ap(),
        "wq": nc.dram_tensor("wq", [D, DHC], BF16, kind="ExternalInput").ap(),
        "wk": nc.dram_tensor("wk", [D, DHC], BF16, kind="ExternalInput").ap(),
        "wv": nc.dram_tensor("wv", [D, DHC], BF16, kind="ExternalInput").ap(),
        "wo": nc.dram_tensor("wo", [DHC, D], BF16, kind="ExternalInput").ap(),
        "out": nc.dram_tensor("out", [S, D], BF16, kind="ExternalOutput").ap(),
    }
    with tile.TileContext(nc) as tc:
        with ExitStack() as ctx:
            _build_body(ctx, tc, io)
    nc.finalize()
    return nc


_NC = None


def _get_nc():
    global _NC
    if _NC is None:
        _NC = build_nc()
    return _NC


def make_in_maps(x, Wq, Wk, Wv, Wo):
    bf = ml_dtypes.bfloat16
    x = np.asarray(x, dtype=np.float32)
    in_maps = []
    xbs = []
    for b in range(B):
        xT = np.ascontiguousarray(x[b].T).astype(bf)            # [D, S]
        blk = xT.reshape(KC, P, NQG, QG).transpose(2, 0, 1, 3)  # [j, k, p, e]
        xbs.append(np.ascontiguousarray(blk))
    for c in range(NCORES):
        b, g = divmod(c, 4)
        sl = slice(DHC * g, DHC * (g + 1))
        in_maps.append({
            "xb": xbs[b],
            "wq": np.ascontiguousarray(np.asarray(Wq, np.float32)[:, sl]).astype(bf),
            "wk": np.ascontiguousarray(np.asarray(Wk, np.float32)[:, sl]).astype(bf),
            "wv": np.ascontiguousarray(np.asarray(Wv, np.float32)[:, sl]).astype(bf),
            "wo": np.ascontiguousarray(np.asarray(Wo, np.float32)[sl, :]).astype(bf),
        })
    return in_maps


def run(in_maps, trace=False, **kw):
    return run_bass_kernel_spmd(_get_nc(), in_maps, list(range(NCORES)),
                                trace=trace, **kw)


def kernel(x, Wq, Wk, Wv, Wo, bo):
    res = run(make_in_maps(x, Wq, Wk, Wv, Wo)).results
    bo = np.asarray(bo, np.float32)
    out = np.empty((B, S, D), np.float32)
    for b in range(B):
        acc = res[4 * b]["out"].astype(np.float32)
        for g in range(1, 4):
            acc = acc + res[4 * b + g]["out"].astype(np.float32)
        out[b] = acc + bo[None, :]
    return out


# revision 44
# speedup vs baseline: 1.0126x; 1.0126x over previous
"""Multi-head causal attention (B=2, S=2048, D=1024, H=16, hd=64) on 8 trn2 cores.

Sharding: core c handles batch b = c//4 and head-group g = c%4 (heads 4g..4g+4,
d-slice 256g..256g+256 of the QKV projections / Wo rows).  Each core computes a
partial out-projection [2048, 1024] in bf16; the host sums the 4 head-group
partials per batch in f32 and adds the bias.

Per-core kernel (all matmuls bf16, accumulate f32 in PSUM):
  qT/kT = (x @ Wq/k)^T computed directly as [256, 2048] via lhsT=W chunks.
  v     = x @ Wv in natural [seq, head, 66] layout (cols 64/65 = 1.0 so the
          attention rowsum falls out of the ctx matmul as row 64).
  S^T   = k_h @ q_h^T  [kpos, qpos] tiles; exp via ACT (scale=1/8) PSUM->SBUF;
          causal = skip invalid column blocks + triangular bf16 mask on
          diagonal blocks.
  ctx~T = v'_h^T @ expS^T accumulated over kpos blocks -> [66, 512] PSUM
          (row 64 = softmax denominator).
  out  += (ctx~T / rowsum)^T @ Wo rows (normalization: approx reciprocal +
          SBUF->SBUF partition-broadcast DMA + DVE multiply, with the
          multiplies emitted at the start of the following group so the DMA
          latency never blocks the DVE queue).

Scheduling: host prepacks x into contiguous [j][k][128, 512] blocks so each of
the 4 column-group DMAs is one big issue landing j=0 first; weights are one
issue each.  Emission is software-pipelined: attention groups run in ascending
size right behind their projections, with projection / out-projection matmuls
pulled from a fill queue between each score->exp->ctx step so the PE never
waits on the ACT exp.  PSUM->SBUF copies are spread over DVE and GpSimd; ACT
does only exp.
"""

import sys

import numpy as np

for _p in ("/opt/trn_rl_repo",):
    if _p not in sys.path:
        sys.path.insert(0, _p)

import ml_dtypes

import concourse.bass as bass
import concourse.mybir as mybir
import concourse.tile as tile
from concourse import bacc
from concourse.bass_utils import run_bass_kernel_spmd
from concourse.masks import make_upper_triangular

BF16 = mybir.dt.bfloat16
F32 = mybir.dt.float32

import os

NRM_PE = os.environ.get("K_NRM_PE", "1") == "1"      # PE-broadcast normalize
# NOTE: K-split ctx pairs hang real hardware (a PSUM accumulation group may
# not span two PE tile positions), though CoreSim/TimelineSim accept them.
CTX_SPLIT = os.environ.get("K_CTX_SPLIT", "0") == "1"  # K-split paired ctx
W_ACT_RING = os.environ.get("K_W_ACT", "1") == "1"   # weights on ACT hwdge ring
SP_TAIL = os.environ.get("K_SP_TAIL", "1") == "1"    # tail outproj via spool

B, S, D, H, HD = 2, 2048, 1024, 16, 64
NCORES = 8
HPC = 4          # heads per core
DHC = HPC * HD   # 256: d-slice per core
P = 128
SB = S // P      # 16 seq blocks
KC = D // P      # 8 contraction chunks for projections
QG = 512         # q column group width
NQG = S // QG    # 4
VW = HD + 2      # 66: v cols per head (64 data + 2 ones; even M for PE)


def _build_body(ctx, tc, io):
    nc = tc.nc
    xb, wq, wk, wv, wo, out = (
        io["xb"], io["wq"], io["wk"], io["wv"], io["wo"], io["out"],
    )

    consts = ctx.enter_context(tc.tile_pool(name="consts", bufs=1))
    persist = ctx.enter_context(tc.tile_pool(name="persist", bufs=1))
    spool = ctx.enter_context(tc.tile_pool(name="spsum", bufs=2, space="PSUM"))
    cxpool = ctx.enter_context(tc.tile_pool(name="cxpsum", bufs=2, space="PSUM"))
    pjpool = ctx.enter_context(tc.tile_pool(name="pjpsum", bufs=2, space="PSUM"))
    espool = ctx.enter_context(tc.tile_pool(name="es", bufs=6))
    nrmpool = ctx.enter_context(tc.tile_pool(name="nrm", bufs=4))
    outpool = ctx.enter_context(tc.tile_pool(name="outsb", bufs=3))

    # triangular keep-mask for diagonal blocks: tri[i, j] = 1.0 iff j >= i
    tri = consts.tile([P, P], BF16, tag="tri", name="tri")
    make_upper_triangular(nc, tri[:], val=1.0, diag=True)

    # [1, 64] ones: lhsT of the rowsum partition-broadcast matmul
    ones_l = consts.tile([1, HD], BF16, tag="ones", name="ones")
    nc.vector.memset(ones_l[:], 1.0)

    if not NRM_PE:
        drampool = ctx.enter_context(tc.tile_pool(name="dram", bufs=1,
                                                  space="DRAM"))
        rsc = drampool.tile([16, QG], F32, tag="rsc", name="rsc")

    # preload the ACT exp table so the first real exp doesn't pay the load
    dumm = consts.tile([1, 2], F32, tag="dumm", name="dumm")
    nc.vector.memset(dumm[:], 0.0)
    nc.scalar.activation(dumm[:], dumm[:],
                         mybir.ActivationFunctionType.Exp, scale=0.125)

    # ---- persistent SBUF tensors ----
    # x column-groups: xt[j] holds all 8 k-chunks of q-columns j*512..+512
    xt = [persist.tile([P, KC, QG], BF16, tag=f"xt{j}", name=f"xt{j}")
          for j in range(NQG)]
    wq_sb = persist.tile([P, KC, DHC], BF16, tag="wq", name="wq")
    wk_sb = persist.tile([P, KC, DHC], BF16, tag="wk", name="wk")
    wv_sb = persist.tile([P, KC, DHC], BF16, tag="wv", name="wv")
    wo_sb = persist.tile([P, 2, D], BF16, tag="wo", name="wo")
    v_sb = [persist.tile([P, HPC, VW], BF16, tag=f"v{s}", name=f"v{s}")
            for s in range(SB)]
    qt = [persist.tile([P, S], BF16, tag=f"qt{i}", name=f"qt{i}") for i in range(2)]
    kt = [persist.tile([P, S], BF16, tag=f"kt{i}", name=f"kt{i}") for i in range(2)]
    ctxT = [persist.tile([P, S], BF16, tag=f"ctxT{i}", name=f"ctxT{i}")
            for i in range(2)]

    # v ones columns are constant: set once up front (cols 64,65 both 1.0;
    # col 65 is never read, it only keeps M even for the PE)
    for s in range(SB):
        nc.vector.memset(v_sb[s][:, :, HD:VW], 1.0)

    # ---- input DMAs ----
    # All inputs are host-prepacked in exactly the SBUF layout (partition-
    # major), so each transfer is a linear copy with 4-8KB per-partition
    # descriptors.  Weights on the Activation hwdge ring, x on the SP ring;
    # only first-needed transfers are issued up front — the rest go on the
    # ACT ring behind the first v-proj copy, whose data dependency stalls
    # that queue until the first matmuls run, so they can't steal DMA
    # bandwidth from the critical wv/wq/wk/xt0 tiles.
    weng = nc.scalar if W_ACT_RING else nc.sync
    weng.dma_start(out=wv_sb[:], in_=wv[:])
    nc.sync.dma_start(out=xt[0][:], in_=xb[0])
    weng.dma_start(out=wq_sb[:], in_=wq[:])
    weng.dma_start(out=wk_sb[:], in_=wk[:])

    def emit_late_dmas():
        nc.scalar.dma_start(out=xt[1][:], in_=xb[1])
        nc.scalar.dma_start(out=xt[2][:], in_=xb[2])
        nc.scalar.dma_start(out=xt[3][:], in_=xb[3])
        nc.scalar.dma_start(out=wo_sb[:], in_=wo[:])

    # ---- fill-unit generators (each yield = ~1-2 PE matmuls emitted) ----
    def gen_v(sv):
        # seq blocks s=2sv, 2sv+1 -> v natural layout
        for par in range(2):
            s = 2 * sv + par
            j, sj = divmod(s, 4)
            ps = pjpool.tile([P, QG], F32, tag="pj", name="pj")
            for k in range(KC):
                nc.tensor.matmul(
                    ps[:, 0:DHC],
                    lhsT=xt[j][:, k, sj * P:(sj + 1) * P],
                    rhs=wv_sb[:, k, :],
                    start=(k == 0),
                    stop=(k == KC - 1),
                )
                if k % 2 == 1:
                    yield
            src_ap = ps[:, 0:DHC].rearrange("p (h d) -> p h d", h=HPC)
            if par == 0:
                nc.scalar.copy(v_sb[s][:, :, 0:HD], src_ap)
            else:
                nc.vector.tensor_copy(v_sb[s][:, :, 0:HD], src_ap)
            yield

    def gen_qk(pair, j):
        # q and k projections for d-chunk `pair`, q column group j
        for w_sb, dst, use_act in ((wq_sb, qt, False), (wk_sb, kt, True)):
            ps = pjpool.tile([P, QG], F32, tag="pj", name="pj")
            for k in range(KC):
                nc.tensor.matmul(
                    ps[:],
                    lhsT=w_sb[:, k, pair * P:(pair + 1) * P],
                    rhs=xt[j][:, k, :],
                    start=(k == 0),
                    stop=(k == KC - 1),
                )
                if k % 2 == 1:
                    yield
            if use_act:
                nc.scalar.copy(dst[pair][:, j * QG:(j + 1) * QG], ps[:])
            else:
                nc.vector.tensor_copy(dst[pair][:, j * QG:(j + 1) * QG], ps[:])
            yield

    def gen_outproj(m, use_sp=False):
        ot = outpool.tile([P, D], BF16, tag="ot", name="ot")
        if use_sp:
            # tail variant: attention is done, so use the (free) score pool —
            # its 2 banks hold both 512-col halves, and bufs=2 pipelines
            # consecutive m's
            ps = spool.tile([P, 2, QG], F32, tag="sp", name="sp")
            for kc in range(2):
                mb = ctxT[kc][:, m * P:(m + 1) * P]
                st, sp_ = (kc == 0), (kc == 1)
                nc.tensor.matmul(ps[:, 0, :], lhsT=mb,
                                 rhs=wo_sb[:, kc, 0:QG], start=st, stop=sp_)
                nc.tensor.matmul(ps[:, 1, :], lhsT=mb,
                                 rhs=wo_sb[:, kc, QG:D], start=st, stop=sp_)
                yield
            nc.scalar.copy(ot[:, 0:QG], ps[:, 0, :])
            nc.vector.tensor_copy(ot[:, QG:D], ps[:, 1, :])
            yield
        else:
            for n2 in range(2):
                ps = pjpool.tile([P, QG], F32, tag="pj", name="pj")
                for kc in range(2):
                    nc.tensor.matmul(
                        ps[:],
                        lhsT=ctxT[kc][:, m * P:(m + 1) * P],
                        rhs=wo_sb[:, kc, n2 * QG:(n2 + 1) * QG],
                        start=(kc == 0),
                        stop=(kc == 1),
                    )
                yield
                nc.vector.tensor_copy(ot[:, n2 * QG:(n2 + 1) * QG], ps[:])
                yield
        nc.sync.dma_start(out=out[m * P:(m + 1) * P, :], in_=ot[:])
        yield

    # ---- fill queue machinery (purely an emission-order device) ----
    queue = []           # list of (name, generator)
    done = set()

    def push(name, gen):
        queue.append((name, gen))

    def pull(n):
        emitted = 0
        while emitted < n and queue:
            name, gen = queue[0]
            try:
                next(gen)
                emitted += 1
            except StopIteration:
                done.add(name)
                queue.pop(0)

    def drain_until(name):
        # emit exactly up to the end of generator `name` (not the whole queue)
        while name not in done:
            if not queue:
                raise RuntimeError(f"drain_until({name}): not queued")
            pull(1)

    def finish_head():
        # run the head generator to completion so no fill generator holds an
        # in-flight pjpool tile (bufs=1) when we emit pj work directly —
        # interleaving would deadlock the PE queue on the shared bank
        if queue:
            name = queue[0][0]
            while queue and queue[0][0] == name:
                pull(1)

    # deferred softmax-normalize multiplies: (pair, g, cxs, rb) from the
    # previous attention group, emitted shortly into the next one (gives the
    # broadcast DMA time to land without blocking the DVE queue); finished
    # out-projection inputs are queued as fill right after
    pending_nrm = [None]

    def emit_pending_nrm():
        if pending_nrm[0] is None:
            return
        pair, g, cxs, rsb = pending_nrm[0]
        pending_nrm[0] = None
        rbr = nrmpool.tile([HD, 2, QG], F32, tag="rbr", name="rbr")
        if NRM_PE:
            finish_head()
            for hh in range(2):
                # broadcast the bf16 rowsum over 64 partitions via a K=1
                # matmul, then reciprocal straight off PSUM and normalize
                bc = pjpool.tile([P, QG], F32, tag="pj", name="pj")
                nc.tensor.matmul(bc[0:HD, :], lhsT=ones_l[:], rhs=rsb[hh][:],
                                 start=True, stop=True)
                nc.vector.reciprocal_approx_fast(rbr[:, hh, :], bc[0:HD, :])
                nc.vector.tensor_mul(
                    ctxT[pair][hh * HD:(hh + 1) * HD, g * QG:(g + 1) * QG],
                    cxs[hh][0:HD, :],
                    rbr[:, hh, :],
                )
        else:
            rb = rsb  # [HD, 2, QG] broadcast rowsums read back from DRAM
            nc.vector.reciprocal_approx_fast(rbr[:], rb[:])
            for hh in range(2):
                nc.vector.tensor_mul(
                    ctxT[pair][hh * HD:(hh + 1) * HD, g * QG:(g + 1) * QG],
                    cxs[hh][0:HD, :],
                    rbr[:, hh, :],
                )
        if pair == 1:
            for m in range(4 * g, 4 * g + 4):
                push(f"op{m}", gen_outproj(m, use_sp=(g == 3 and SP_TAIL)))

    # ---- attention group ----
    def emit_attention_group(pair, g, npull):
        cxs = [cxpool.tile([VW, QG], F32, tag="cx", name="cx") for _ in range(2)]
        nkb = 4 * g + 4
        sp_tiles = {}

        def emit_score(kb):
            c0 = P * (kb - 4 * g) if kb >= 4 * g else 0
            sp_t = spool.tile([P, 2, QG], F32, tag="sp", name="sp")
            for hh in range(2):
                nc.tensor.matmul(
                    sp_t[:, hh, c0:QG],
                    lhsT=kt[pair][hh * HD:(hh + 1) * HD, kb * P:(kb + 1) * P],
                    rhs=qt[pair][hh * HD:(hh + 1) * HD, g * QG + c0:(g + 1) * QG],
                    start=True,
                    stop=True,
                )
            sp_tiles[kb] = (sp_t, c0)

        emit_score(0)
        for kb in range(nkb):
            sp_t, c0 = sp_tiles.pop(kb)
            es_t = espool.tile([P, 2, QG], BF16, tag="es", name="es")
            nc.scalar.activation(
                es_t[:, :, c0:QG], sp_t[:, :, c0:QG],
                mybir.ActivationFunctionType.Exp, scale=0.125,
            )
            if kb >= 4 * g:
                dst = es_t[:, :, c0:c0 + P]
                t_ap = tri[:]
                tri_b = bass.AP(t_ap.tensor, t_ap.offset,
                                [t_ap.ap[0], [0, 2], t_ap.ap[1]])
                nc.vector.tensor_mul(dst, dst, tri_b)
            if kb == 0:
                emit_pending_nrm()
            if kb + 1 < nkb:
                emit_score(kb + 1)
            pull(npull)
            st = (kb == 0)
            sp_ = (kb == nkb - 1)
            h0, h1 = 2 * pair, 2 * pair + 1
            if CTX_SPLIT:
                # K split in 64-row halves so the two heads run concurrently
                # on the two PE row-quadrants (different PSUM banks)
                nc.tensor.matmul(cxs[0][:, c0:QG], lhsT=v_sb[kb][0:HD, h0, :],
                                 rhs=es_t[0:HD, 0, c0:QG], start=st, stop=False)
                nc.tensor.matmul(cxs[1][:, c0:QG], lhsT=v_sb[kb][HD:P, h1, :],
                                 rhs=es_t[HD:P, 1, c0:QG], start=st, stop=False)
                nc.tensor.matmul(cxs[0][:, c0:QG], lhsT=v_sb[kb][HD:P, h0, :],
                                 rhs=es_t[HD:P, 0, c0:QG], start=False, stop=sp_)
                nc.tensor.matmul(cxs[1][:, c0:QG], lhsT=v_sb[kb][0:HD, h1, :],
                                 rhs=es_t[0:HD, 1, c0:QG], start=False, stop=sp_)
            else:
                for hh in range(2):
                    h = 2 * pair + hh
                    nc.tensor.matmul(
                        cxs[hh][:, c0:QG],
                        lhsT=v_sb[kb][:, h, :],
                        rhs=es_t[:, hh, c0:QG],
                        start=st,
                        stop=sp_,
                    )
        # group end: stage the rowsum rows; broadcast + reciprocal +
        # normalize deferred to the next group
        if NRM_PE:
            rsb = [nrmpool.tile([1, QG], BF16, tag=f"rs{hh}", name=f"rs{hh}")
                   for hh in range(2)]
            for hh in range(2):
                nc.vector.tensor_copy(rsb[hh][:], cxs[hh][HD:HD + 1, :])
            pending_nrm[0] = (pair, g, cxs, rsb)
        else:
            slot = (pair * NQG + g) * 2
            for hh in range(2):
                rs = nrmpool.tile([1, QG], F32, tag=f"rs{hh}", name=f"rs{hh}")
                nc.vector.tensor_copy(rs[:], cxs[hh][HD:HD + 1, :])
                nc.sync.dma_start(out=rsc[slot + hh:slot + hh + 1, :],
                                  in_=rs[:])
            rb = nrmpool.tile([HD, 2, QG], F32, tag="rb", name="rb")
            sl = rsc[slot:slot + 2, :]
            src = bass.AP(sl.tensor, sl.offset, [[0, HD], [QG, 2], [1, QG]])
            nc.sync.dma_start(out=rb[:], in_=src)
            pending_nrm[0] = (pair, g, cxs, rb)

    # ---- emission schedule ----
    # v projections for the first column group, then qk(0,0), then attention
    # groups in ascending size with everything else pulled in as filler.
    for _ in gen_v(0):
        pass
    emit_late_dmas()
    for _ in gen_v(1):
        pass
    for _ in gen_qk(0, 0):
        pass
    for j in range(1, NQG):
        push(f"v{2 * j}", gen_v(2 * j))
        push(f"v{2 * j + 1}", gen_v(2 * j + 1))
        push(f"qk0{j}", gen_qk(0, j))
    for j in range(NQG):
        push(f"qk1{j}", gen_qk(1, j))

    for g in range(NQG):
        if g > 0:
            drain_until(f"qk0{g}")
        emit_attention_group(0, g, npull=3)
    for g in range(NQG):
        drain_until(f"qk1{g}")
        emit_attention_group(1, g, npull=4)
    emit_pending_nrm()
    pull(1 << 30)


def build_nc():
    from contextlib import ExitStack

    nc = bacc.Bacc()
    io = {
        "xb": nc.dram_tensor("xb", [NQG, P, KC, QG], BF16,
                             kind="ExternalInput").ap(),
        "wq": nc.dram_tensor("wq", [P, KC, DHC], BF16,
                             kind="ExternalInput").ap(),
        "wk": nc.dram_tensor("wk", [P, KC, DHC], BF16,
                             kind="ExternalInput").ap(),
        "wv": nc.dram_tensor("wv", [P, KC, DHC], BF16,
                             kind="ExternalInput").ap(),
        "wo": nc.dram_tensor("wo", [P, 2, D], BF16,
                             kind="ExternalInput").ap(),
        "out": nc.dram_tensor("out", [S, D], BF16, kind="ExternalOutput").ap(),
    }
    with tile.TileContext(nc) as tc:
        with ExitStack() as ctx:
            _build_body(ctx, tc, io)
    nc.finalize()
    return nc


_NC = None


def _get_nc():
    global _NC
    if _NC is None:
        _NC = build_nc()
    return _NC


def make_in_maps(x, Wq, Wk, Wv, Wo):
    bf = ml_dtypes.bfloat16
    x = np.asarray(x, dtype=np.float32)

    def pack_w(w, sl):
        # [D, DHC] slice -> partition-major [P, KC, DHC] (SBUF layout)
        w = np.asarray(w, np.float32)[:, sl].astype(bf)
        return np.ascontiguousarray(w.reshape(KC, P, DHC).transpose(1, 0, 2))

    def pack_wo(w, sl):
        # [DHC, D] slice -> partition-major [P, 2, D]
        w = np.asarray(w, np.float32)[sl, :].astype(bf)
        return np.ascontiguousarray(w.reshape(2, P, D).transpose(1, 0, 2))

    in_maps = []
    xbs = []
    for b in range(B):
        xT = np.ascontiguousarray(x[b].T).astype(bf)            # [D, S]
        blk = xT.reshape(KC, P, NQG, QG).transpose(2, 1, 0, 3)  # [j, p, k, e]
        xbs.append(np.ascontiguousarray(blk))
    for c in range(NCORES):
        b, g = divmod(c, 4)
        sl = slice(DHC * g, DHC * (g + 1))
        in_maps.append({
            "xb": xbs[b],
            "wq": pack_w(Wq, sl),
            "wk": pack_w(Wk, sl),
            "wv": pack_w(Wv, sl),
            "wo": pack_wo(Wo, sl),
        })
    return in_maps


def run(in_maps, trace=False, **kw):
    return run_bass_kernel_spmd(_get_nc(), in_maps, list(range(NCORES)),
                                trace=trace, **kw)


def kernel(x, Wq, Wk, Wv, Wo, bo):
    res = run(make_in_maps(x, Wq, Wk, Wv, Wo)).results
    bo = np.asarray(bo, np.float32)
    out = np.empty((B, S, D), np.float32)
    for b in range(B):
        acc = res[4 * b]["out"].astype(np.float32)
        for g in range(1, 4):
            acc = acc + res[4 * b + g]["out"].astype(np.float32)
        out[b] = acc + bo[None, :]
    return out


# revision 46
# speedup vs baseline: 1.0158x; 1.0032x over previous
"""Multi-head causal attention (B=2, S=2048, D=1024, H=16, hd=64) on 8 trn2 cores.

Sharding: core c handles batch b = c//4 and head-group g = c%4 (heads 4g..4g+4,
d-slice 256g..256g+256 of the QKV projections / Wo rows).  Each core computes a
partial out-projection [2048, 1024] in bf16; the host sums the 4 head-group
partials per batch in f32 and adds the bias.

Per-core kernel (all matmuls bf16, accumulate f32 in PSUM):
  qT/kT = (x @ Wq/k)^T computed directly as [256, 2048] via lhsT=W chunks.
  v     = x @ Wv in natural [seq, head, 66] layout (cols 64/65 = 1.0 so the
          attention rowsum falls out of the ctx matmul as row 64).
  S^T   = k_h @ q_h^T  [kpos, qpos] tiles; exp via ACT (scale=1/8) PSUM->SBUF;
          causal = skip invalid column blocks + triangular bf16 mask on
          diagonal blocks.
  ctx~T = v'_h^T @ expS^T accumulated over kpos blocks -> [66, 512] PSUM
          (row 64 = softmax denominator).
  out  += (ctx~T / rowsum)^T @ Wo rows (normalization: approx reciprocal +
          SBUF->SBUF partition-broadcast DMA + DVE multiply, with the
          multiplies emitted at the start of the following group so the DMA
          latency never blocks the DVE queue).

Scheduling: host prepacks x into contiguous [j][k][128, 512] blocks so each of
the 4 column-group DMAs is one big issue landing j=0 first; weights are one
issue each.  Emission is software-pipelined: attention groups run in ascending
size right behind their projections, with projection / out-projection matmuls
pulled from a fill queue between each score->exp->ctx step so the PE never
waits on the ACT exp.  PSUM->SBUF copies are spread over DVE and GpSimd; ACT
does only exp.
"""

import sys

import numpy as np

for _p in ("/opt/trn_rl_repo",):
    if _p not in sys.path:
        sys.path.insert(0, _p)

import ml_dtypes

import concourse.bass as bass
import concourse.mybir as mybir
import concourse.tile as tile
from concourse import bacc
from concourse.bass_utils import run_bass_kernel_spmd
from concourse.masks import make_upper_triangular

BF16 = mybir.dt.bfloat16
F32 = mybir.dt.float32

import os

NRM_PE = os.environ.get("K_NRM_PE", "1") == "1"      # PE-broadcast normalize
# NOTE: K-split ctx pairs hang real hardware (a PSUM accumulation group may
# not span two PE tile positions), though CoreSim/TimelineSim accept them.
CTX_SPLIT = os.environ.get("K_CTX_SPLIT", "0") == "1"  # K-split paired ctx
W_ACT_RING = os.environ.get("K_W_ACT", "1") == "1"   # weights on ACT hwdge ring
SP_TAIL = os.environ.get("K_SP_TAIL", "1") == "1"    # tail outproj via spool

B, S, D, H, HD = 2, 2048, 1024, 16, 64
NCORES = 8
HPC = 4          # heads per core
DHC = HPC * HD   # 256: d-slice per core
P = 128
SB = S // P      # 16 seq blocks
KC = D // P      # 8 contraction chunks for projections
QG = 512         # q column group width
NQG = S // QG    # 4
VW = HD + 2      # 66: v cols per head (64 data + 2 ones; even M for PE)


def _build_body(ctx, tc, io):
    nc = tc.nc
    xb, wq, wk, wv, wo, out = (
        io["xb"], io["wq"], io["wk"], io["wv"], io["wo"], io["out"],
    )

    consts = ctx.enter_context(tc.tile_pool(name="consts", bufs=1))
    persist = ctx.enter_context(tc.tile_pool(name="persist", bufs=1))
    spool = ctx.enter_context(tc.tile_pool(name="spsum", bufs=2, space="PSUM"))
    cxpool = ctx.enter_context(tc.tile_pool(name="cxpsum", bufs=2, space="PSUM"))
    pjpool = ctx.enter_context(tc.tile_pool(name="pjpsum", bufs=2, space="PSUM"))
    espool = ctx.enter_context(tc.tile_pool(name="es", bufs=6))
    nrmpool = ctx.enter_context(tc.tile_pool(name="nrm", bufs=4))
    outpool = ctx.enter_context(tc.tile_pool(name="outsb", bufs=3))

    # triangular keep-mask for diagonal blocks: tri[i, j] = 1.0 iff j >= i
    tri = consts.tile([P, P], BF16, tag="tri", name="tri")
    make_upper_triangular(nc, tri[:], val=1.0, diag=True)

    # [1, 64] ones: lhsT of the rowsum partition-broadcast matmul
    ones_l = consts.tile([1, HD], BF16, tag="ones", name="ones")
    nc.vector.memset(ones_l[:], 1.0)

    if not NRM_PE:
        drampool = ctx.enter_context(tc.tile_pool(name="dram", bufs=1,
                                                  space="DRAM"))
        rsc = drampool.tile([16, QG], F32, tag="rsc", name="rsc")

    # preload the ACT exp table so the first real exp doesn't pay the load
    dumm = consts.tile([1, 2], F32, tag="dumm", name="dumm")
    nc.vector.memset(dumm[:], 0.0)
    nc.scalar.activation(dumm[:], dumm[:],
                         mybir.ActivationFunctionType.Exp, scale=0.125)

    # ---- persistent SBUF tensors ----
    # x column-groups: xt[j] holds all 8 k-chunks of q-columns j*512..+512
    xt = [persist.tile([P, KC, QG], BF16, tag=f"xt{j}", name=f"xt{j}")
          for j in range(NQG)]
    wq_sb = persist.tile([P, KC, DHC], BF16, tag="wq", name="wq")
    wk_sb = persist.tile([P, KC, DHC], BF16, tag="wk", name="wk")
    wv_sb = persist.tile([P, KC, DHC], BF16, tag="wv", name="wv")
    wo_sb = persist.tile([P, 2, D], BF16, tag="wo", name="wo")
    v_sb = [persist.tile([P, HPC, VW], BF16, tag=f"v{s}", name=f"v{s}")
            for s in range(SB)]
    qt = [persist.tile([P, S], BF16, tag=f"qt{i}", name=f"qt{i}") for i in range(2)]
    kt = [persist.tile([P, S], BF16, tag=f"kt{i}", name=f"kt{i}") for i in range(2)]
    ctxT = [persist.tile([P, S], BF16, tag=f"ctxT{i}", name=f"ctxT{i}")
            for i in range(2)]

    # v ones columns are constant: set once up front (cols 64,65 both 1.0;
    # col 65 is never read, it only keeps M even for the PE)
    for s in range(SB):
        nc.vector.memset(v_sb[s][:, :, HD:VW], 1.0)

    # ---- input DMAs ----
    # All inputs are host-prepacked in exactly the SBUF layout (partition-
    # major), so each transfer is a linear copy with 4-8KB per-partition
    # descriptors.  Weights on the Activation hwdge ring, x on the SP ring;
    # only first-needed transfers are issued up front — the rest go on the
    # ACT ring behind the first v-proj copy, whose data dependency stalls
    # that queue until the first matmuls run, so they can't steal DMA
    # bandwidth from the critical wv/wq/wk/xt0 tiles.
    weng = nc.scalar if W_ACT_RING else nc.sync
    weng.dma_start(out=wv_sb[:], in_=wv[:])
    nc.sync.dma_start(out=xt[0][:], in_=xb[0])
    weng.dma_start(out=wq_sb[:], in_=wq[:])
    weng.dma_start(out=wk_sb[:], in_=wk[:])

    def emit_late_dmas():
        nc.scalar.dma_start(out=xt[1][:], in_=xb[1])
        nc.scalar.dma_start(out=xt[2][:], in_=xb[2])
        nc.scalar.dma_start(out=xt[3][:], in_=xb[3])
        nc.scalar.dma_start(out=wo_sb[:], in_=wo[:])

    # ---- fill-unit generators (each yield = ~1-2 PE matmuls emitted) ----
    def gen_v(sv):
        # seq blocks s=2sv, 2sv+1 -> v natural layout
        for par in range(2):
            s = 2 * sv + par
            j, sj = divmod(s, 4)
            ps = pjpool.tile([P, QG], F32, tag="pj", name="pj")
            for k in range(KC):
                nc.tensor.matmul(
                    ps[:, 0:DHC],
                    lhsT=xt[j][:, k, sj * P:(sj + 1) * P],
                    rhs=wv_sb[:, k, :],
                    start=(k == 0),
                    stop=(k == KC - 1),
                )
                if k % 2 == 1:
                    yield
            src_ap = ps[:, 0:DHC].rearrange("p (h d) -> p h d", h=HPC)
            if par == 0:
                nc.scalar.copy(v_sb[s][:, :, 0:HD], src_ap)
            else:
                nc.vector.tensor_copy(v_sb[s][:, :, 0:HD], src_ap)
            yield

    def gen_qk(pair, j):
        # q and k projections for d-chunk `pair`, q column group j
        for w_sb, dst, use_act in ((wq_sb, qt, False), (wk_sb, kt, True)):
            ps = pjpool.tile([P, QG], F32, tag="pj", name="pj")
            for k in range(KC):
                nc.tensor.matmul(
                    ps[:],
                    lhsT=w_sb[:, k, pair * P:(pair + 1) * P],
                    rhs=xt[j][:, k, :],
                    start=(k == 0),
                    stop=(k == KC - 1),
                )
                if k % 2 == 1:
                    yield
            if use_act:
                nc.scalar.copy(dst[pair][:, j * QG:(j + 1) * QG], ps[:])
            else:
                nc.vector.tensor_copy(dst[pair][:, j * QG:(j + 1) * QG], ps[:])
            yield

    def gen_outproj(m, use_sp=False):
        ot = outpool.tile([P, D], BF16, tag="ot", name="ot")
        if use_sp:
            # tail variant: attention is done, so use the (free) score pool —
            # its 2 banks hold both 512-col halves, and bufs=2 pipelines
            # consecutive m's
            ps = spool.tile([P, 2, QG], F32, tag="sp", name="sp")
            for kc in range(2):
                mb = ctxT[kc][:, m * P:(m + 1) * P]
                st, sp_ = (kc == 0), (kc == 1)
                nc.tensor.matmul(ps[:, 0, :], lhsT=mb,
                                 rhs=wo_sb[:, kc, 0:QG], start=st, stop=sp_)
                nc.tensor.matmul(ps[:, 1, :], lhsT=mb,
                                 rhs=wo_sb[:, kc, QG:D], start=st, stop=sp_)
                yield
            nc.scalar.copy(ot[:, 0:QG], ps[:, 0, :])
            nc.vector.tensor_copy(ot[:, QG:D], ps[:, 1, :])
            yield
        else:
            for n2 in range(2):
                ps = pjpool.tile([P, QG], F32, tag="pj", name="pj")
                for kc in range(2):
                    nc.tensor.matmul(
                        ps[:],
                        lhsT=ctxT[kc][:, m * P:(m + 1) * P],
                        rhs=wo_sb[:, kc, n2 * QG:(n2 + 1) * QG],
                        start=(kc == 0),
                        stop=(kc == 1),
                    )
                yield
                nc.vector.tensor_copy(ot[:, n2 * QG:(n2 + 1) * QG], ps[:])
                yield
        nc.sync.dma_start(out=out[m * P:(m + 1) * P, :], in_=ot[:])
        yield

    # ---- fill queue machinery (purely an emission-order device) ----
    queue = []           # list of (name, generator)
    done = set()

    def push(name, gen):
        queue.append((name, gen))

    def pull(n):
        emitted = 0
        while emitted < n and queue:
            name, gen = queue[0]
            try:
                next(gen)
                emitted += 1
            except StopIteration:
                done.add(name)
                queue.pop(0)

    def drain_until(name):
        # emit exactly up to the end of generator `name` (not the whole queue)
        while name not in done:
            if not queue:
                raise RuntimeError(f"drain_until({name}): not queued")
            pull(1)

    def finish_head():
        # run the head generator to completion so no fill generator holds an
        # in-flight pjpool tile (bufs=1) when we emit pj work directly —
        # interleaving would deadlock the PE queue on the shared bank
        if queue:
            name = queue[0][0]
            while queue and queue[0][0] == name:
                pull(1)

    # deferred softmax-normalize multiplies: (pair, g, cxs, rb) from the
    # previous attention group, emitted shortly into the next one (gives the
    # broadcast DMA time to land without blocking the DVE queue); finished
    # out-projection inputs are queued as fill right after
    pending_nrm = [None]

    def emit_pending_nrm():
        if pending_nrm[0] is None:
            return
        pair, g, cxs, rsb = pending_nrm[0]
        pending_nrm[0] = None
        rbr = nrmpool.tile([HD, 2, QG], F32, tag="rbr", name="rbr")
        if NRM_PE:
            finish_head()
            for hh in range(2):
                # broadcast the bf16 rowsum over 64 partitions via a K=1
                # matmul, then reciprocal straight off PSUM and normalize
                bc = pjpool.tile([P, QG], F32, tag="pj", name="pj")
                nc.tensor.matmul(bc[0:HD, :], lhsT=ones_l[:], rhs=rsb[hh][:],
                                 start=True, stop=True)
                nc.vector.reciprocal_approx_fast(rbr[:, hh, :], bc[0:HD, :])
                nc.vector.tensor_mul(
                    ctxT[pair][hh * HD:(hh + 1) * HD, g * QG:(g + 1) * QG],
                    cxs[hh][0:HD, :],
                    rbr[:, hh, :],
                )
        else:
            rb = rsb  # [HD, 2, QG] broadcast rowsums read back from DRAM
            nc.vector.reciprocal_approx_fast(rbr[:], rb[:])
            for hh in range(2):
                nc.vector.tensor_mul(
                    ctxT[pair][hh * HD:(hh + 1) * HD, g * QG:(g + 1) * QG],
                    cxs[hh][0:HD, :],
                    rbr[:, hh, :],
                )
        if pair == 1:
            if g == 3 and SP_TAIL:
                # tail: interleave m-pairs so each m's kc=0 matmul (which
                # needs only the long-ready ctxT[0]) prefills the PE while
                # this group's normalize chain completes; pair-wise (not
                # 4-wide) so sp-pool slot reuse stays in program order
                for a, b in ((12, 13), (14, 15)):
                    ga = gen_outproj(a, use_sp=True)
                    gb = gen_outproj(b, use_sp=True)
                    alive = [ga, gb]
                    while alive:
                        for gen in list(alive):
                            try:
                                next(gen)
                            except StopIteration:
                                alive.remove(gen)
            else:
                for m in range(4 * g, 4 * g + 4):
                    push(f"op{m}", gen_outproj(m))

    # ---- attention group ----
    def emit_attention_group(pair, g, npull):
        cxs = [cxpool.tile([VW, QG], F32, tag="cx", name="cx") for _ in range(2)]
        nkb = 4 * g + 4
        sp_tiles = {}

        def emit_score(kb):
            c0 = P * (kb - 4 * g) if kb >= 4 * g else 0
            sp_t = spool.tile([P, 2, QG], F32, tag="sp", name="sp")
            for hh in range(2):
                nc.tensor.matmul(
                    sp_t[:, hh, c0:QG],
                    lhsT=kt[pair][hh * HD:(hh + 1) * HD, kb * P:(kb + 1) * P],
                    rhs=qt[pair][hh * HD:(hh + 1) * HD, g * QG + c0:(g + 1) * QG],
                    start=True,
                    stop=True,
                )
            sp_tiles[kb] = (sp_t, c0)

        emit_score(0)
        for kb in range(nkb):
            sp_t, c0 = sp_tiles.pop(kb)
            es_t = espool.tile([P, 2, QG], BF16, tag="es", name="es")
            nc.scalar.activation(
                es_t[:, :, c0:QG], sp_t[:, :, c0:QG],
                mybir.ActivationFunctionType.Exp, scale=0.125,
            )
            if kb >= 4 * g:
                dst = es_t[:, :, c0:c0 + P]
                t_ap = tri[:]
                tri_b = bass.AP(t_ap.tensor, t_ap.offset,
                                [t_ap.ap[0], [0, 2], t_ap.ap[1]])
                nc.vector.tensor_mul(dst, dst, tri_b)
            if kb == 0:
                emit_pending_nrm()
            if kb + 1 < nkb:
                emit_score(kb + 1)
            pull(npull)
            st = (kb == 0)
            sp_ = (kb == nkb - 1)
            h0, h1 = 2 * pair, 2 * pair + 1
            if CTX_SPLIT:
                # K split in 64-row halves so the two heads run concurrently
                # on the two PE row-quadrants (different PSUM banks)
                nc.tensor.matmul(cxs[0][:, c0:QG], lhsT=v_sb[kb][0:HD, h0, :],
                                 rhs=es_t[0:HD, 0, c0:QG], start=st, stop=False)
                nc.tensor.matmul(cxs[1][:, c0:QG], lhsT=v_sb[kb][HD:P, h1, :],
                                 rhs=es_t[HD:P, 1, c0:QG], start=st, stop=False)
                nc.tensor.matmul(cxs[0][:, c0:QG], lhsT=v_sb[kb][HD:P, h0, :],
                                 rhs=es_t[HD:P, 0, c0:QG], start=False, stop=sp_)
                nc.tensor.matmul(cxs[1][:, c0:QG], lhsT=v_sb[kb][0:HD, h1, :],
                                 rhs=es_t[0:HD, 1, c0:QG], start=False, stop=sp_)
            else:
                for hh in range(2):
                    h = 2 * pair + hh
                    nc.tensor.matmul(
                        cxs[hh][:, c0:QG],
                        lhsT=v_sb[kb][:, h, :],
                        rhs=es_t[:, hh, c0:QG],
                        start=st,
                        stop=sp_,
                    )
        # group end: stage the rowsum rows; broadcast + reciprocal +
        # normalize deferred to the next group
        if NRM_PE:
            rsb = [nrmpool.tile([1, QG], BF16, tag=f"rs{hh}", name=f"rs{hh}")
                   for hh in range(2)]
            for hh in range(2):
                nc.vector.tensor_copy(rsb[hh][:], cxs[hh][HD:HD + 1, :])
            pending_nrm[0] = (pair, g, cxs, rsb)
        else:
            slot = (pair * NQG + g) * 2
            for hh in range(2):
                rs = nrmpool.tile([1, QG], F32, tag=f"rs{hh}", name=f"rs{hh}")
                nc.vector.tensor_copy(rs[:], cxs[hh][HD:HD + 1, :])
                nc.sync.dma_start(out=rsc[slot + hh:slot + hh + 1, :],
                                  in_=rs[:])
            rb = nrmpool.tile([HD, 2, QG], F32, tag="rb", name="rb")
            sl = rsc[slot:slot + 2, :]
            src = bass.AP(sl.tensor, sl.offset, [[0, HD], [QG, 2], [1, QG]])
            nc.sync.dma_start(out=rb[:], in_=src)
            pending_nrm[0] = (pair, g, cxs, rb)

    # ---- emission schedule ----
    # v projections for the first column group, then qk(0,0), then attention
    # groups in ascending size with everything else pulled in as filler.
    for _ in gen_v(0):
        pass
    emit_late_dmas()
    for _ in gen_v(1):
        pass
    for _ in gen_qk(0, 0):
        pass
    for j in range(1, NQG):
        push(f"v{2 * j}", gen_v(2 * j))
        push(f"v{2 * j + 1}", gen_v(2 * j + 1))
        push(f"qk0{j}", gen_qk(0, j))
    for j in range(NQG):
        push(f"qk1{j}", gen_qk(1, j))

    for g in range(NQG):
        if g > 0:
            drain_until(f"qk0{g}")
        emit_attention_group(0, g, npull=3)
    for g in range(NQG):
        drain_until(f"qk1{g}")
        emit_attention_group(1, g, npull=4)
    emit_pending_nrm()
    pull(1 << 30)


def build_nc():
    from contextlib import ExitStack

    nc = bacc.Bacc()
    io = {
        "xb": nc.dram_tensor("xb", [NQG, P, KC, QG], BF16,
                             kind="ExternalInput").ap(),
        "wq": nc.dram_tensor("wq", [P, KC, DHC], BF16,
                             kind="ExternalInput").ap(),
        "wk": nc.dram_tensor("wk", [P, KC, DHC], BF16,
                             kind="ExternalInput").ap(),
        "wv": nc.dram_tensor("wv", [P, KC, DHC], BF16,
                             kind="ExternalInput").ap(),
        "wo": nc.dram_tensor("wo", [P, 2, D], BF16,
                             kind="ExternalInput").ap(),
        "out": nc.dram_tensor("out", [S, D], BF16, kind="ExternalOutput").ap(),
    }
    with tile.TileContext(nc) as tc:
        with ExitStack() as ctx:
            _build_body(ctx, tc, io)
    nc.finalize()
    return nc


_NC = None


def _get_nc():
    global _NC
    if _NC is None:
        _NC = build_nc()
    return _NC


def make_in_maps(x, Wq, Wk, Wv, Wo):
    bf = ml_dtypes.bfloat16
    x = np.asarray(x, dtype=np.float32)

    def pack_w(w, sl):
        # [D, DHC] slice -> partition-major [P, KC, DHC] (SBUF layout)
        w = np.asarray(w, np.float32)[:, sl].astype(bf)
        return np.ascontiguousarray(w.reshape(KC, P, DHC).transpose(1, 0, 2))

    def pack_wo(w, sl):
        # [DHC, D] slice -> partition-major [P, 2, D]
        w = np.asarray(w, np.float32)[sl, :].astype(bf)
        return np.ascontiguousarray(w.reshape(2, P, D).transpose(1, 0, 2))

    in_maps = []
    xbs = []
    for b in range(B):
        xT = np.ascontiguousarray(x[b].T).astype(bf)            # [D, S]
        blk = xT.reshape(KC, P, NQG, QG).transpose(2, 1, 0, 3)  # [j, p, k, e]
        xbs.append(np.ascontiguousarray(blk))
    for c in range(NCORES):
        b, g = divmod(c, 4)
        sl = slice(DHC * g, DHC * (g + 1))
        in_maps.append({
            "xb": xbs[b],
            "wq": pack_w(Wq, sl),
            "wk": pack_w(Wk, sl),
            "wv": pack_w(Wv, sl),
            "wo": pack_wo(Wo, sl),
        })
    return in_maps


def run(in_maps, trace=False, **kw):
    return run_bass_kernel_spmd(_get_nc(), in_maps, list(range(NCORES)),
                                trace=trace, **kw)


def kernel(x, Wq, Wk, Wv, Wo, bo):
    res = run(make_in_maps(x, Wq, Wk, Wv, Wo)).results
    bo = np.asarray(bo, np.float32)
    out = np.empty((B, S, D), np.float32)
    for b in range(B):
        acc = res[4 * b]["out"].astype(np.float32)
        for g in range(1, 4):
            acc = acc + res[4 * b + g]["out"].astype(np.float32)
        out[b] = acc + bo[None, :]
    return out
